# revision 19
# baseline (speedup 1.0000x reference)
"""Trainium2 Bass kernel for nn_CrossCategoryLoss.

loss(row) = sum_t relu(log_a[A_t] + log_b[B_t] - c_t)
  with c_t = log_g[G_t] (pos) or log(1 - exp(log_g[G_t])) (not).

Rewrites (per-row, exact in fp32 up to rounding):
  p_s   = a[A_s] + b[B_s]                (14 distinct pairs)
  S     = ln(sg) - ln(sa) - ln(sb)
  q'_g  = gamma[g] - S     (pos, g in 4..7)
  q'_wg = ln(sg - eg[g]) - S  (not, g in 0..2)
  term  = relu(p - q') = max(p, q') - q'
  loss  = sum_t max(p_t, q'_t) - sum_g n_g q'_g   (max-trick: no relu)

Engine split per [128, R=512] tile (target: every engine under the
~18us/tile DMA floor so the kernel is HBM-bound):
  ACT   : 3x exp (fp16 out), one fused ln over [P,3,R] PSUM sums,
          ln(wp), loss PSUM->SBUF copy                   (~14.5us)
  PE    : 24 identity-matmul sum accumulations, 7 weighted q matmuls,
          36 d-term copies -- software-pipelined one tile behind for
          the loss matmuls so the PE stream rarely stalls (p-state
          stays ramped at 2.4GHz)                        (~15us)
  DVE   : wp, S, q' subs, and the 36 maxes fused into 5 broadcast
          tensor_max instructions (fp16 2x mode)         (~15.5us)
  GpSimd: all 14 pair adds (8 batched instructions, X1 group first so
          the maxes can start early)                     (~16us)
  DMA   : 3x 2MiB loads (g+b on sync ring, a on scalar ring),
          0.25MiB store (scalar ring)                    (~18.4us)

fp16 notes: gamma exps in fp16 are safe because sg is accumulated in
fp32 from the same fp16 addends that wp subtracts.  ln-sums and S are
kept in fp16 (values in (-8, 4), abs err ~2e-3) which is well inside
the 2e-2 rel-err budget and enables DVE 2x mode for S / q'not.

Sharding: pure data-parallel over 8 cores; each core handles B/8 rows.
"""

import numpy as np

import concourse.bass as bass
import concourse.bacc as bacc
import concourse.mybir as mybir
from concourse.tile import TileContext
from concourse.tile_rust import add_dep_helper
from concourse.bass_utils import run_bass_kernel_spmd
from concourse import masks

N_CORES = 8
B = 4194304
B_CORE = B // N_CORES          # 524288 rows per core
P = 128                        # partitions
ROWS_PER_PART = B_CORE // P    # 4096
R = 512                        # rows per partition per tile
N_TILES = ROWS_PER_PART // R   # 8

F32 = mybir.dt.float32
F16 = mybir.dt.float16
AF = mybir.ActivationFunctionType

# Pair slot map (slot -> (alpha_idx, beta_idx)), ordered so the X1 group
# (slots 0-4, terms vs {g4,w1,w2}) is produced by the first 3 GpSimd
# batches, X2 (slots 5-9, vs {g5,w0,w2}) by the next 3, tail (10-13) last:
#  s0(0,4) s1(0,6) s2(2,4) s3(4,2) s4(4,0)
#  s5(1,5) s6(1,6) s7(2,5) s8(5,2) s9(5,1)
#  s10(2,6) s11(6,2) s12(2,7) s13(7,2)
# Batches: (a_start, a_stride, b_start, b_stride, slot_start, n)
_PAIR_BATCHES = [
    (0, 0, 4, 2, 0, 2),    # P1: (0,4)s0 (0,6)s1
    (2, 2, 4, -2, 2, 2),   # P2: (2,4)s2 (4,2)s3
    (4, 0, 0, 0, 4, 1),    # P3: (4,0)s4
    (1, 0, 5, 1, 5, 2),    # P4: (1,5)s5 (1,6)s6
    (2, 3, 5, -3, 7, 2),   # P5: (2,5)s7 (5,2)s8
    (5, 0, 1, 0, 9, 1),    # P6: (5,1)s9
    (2, 4, 6, -4, 10, 2),  # P7: (2,6)s10 (6,2)s11
    (2, 5, 7, -5, 12, 2),  # P8: (2,7)s12 (7,2)s13
]

# q16 slot order: [g4, g5, g6, g7, w0, w1, w2] (pos 0:4 from gamma[4:8];
# not 4:7 from ln(wp) - S). Term-count weights per q slot:
_QSLOT_N = [5.0, 5.0, 2.0, 2.0, 5.0, 5.0, 12.0]
# Weighted q matmuls grouped by weight to minimize stationary swaps:
_QORDER = [(5.0, 0), (5.0, 1), (5.0, 4), (5.0, 5),
           (2.0, 2), (2.0, 3), (12.0, 6)]


def _ap3(t, offset_elems, dims):
    """Build [P, ...dims] AP over tile t with free dims [(stride, n), ...]."""
    if isinstance(t, bass.AP):
        a = t
    else:
        a = t[:, :, :] if len(t.shape) == 3 else t[:, :]
    return bass.AP(tensor=a.tensor, offset=a.offset + offset_elems,
                   ap=[a.ap[0]] + [list(d) for d in dims])


def build_kernel(reps: int = 1) -> bass.Bass:
    nc = bacc.Bacc("TRN2", target_bir_lowering=False, debug=False,
                   num_devices=N_CORES)

    # Restrict the ACT table chooser to one set holding Exp+Ln, so no
    # per-call table reloads (~2.7us each) are emitted.
    _orig_tables = bacc.get_activation_tables

    def _one_set(arch):
        return {
            name: (fns if name == "natural_log_exp_and_others" else set())
            for name, fns in _orig_tables(arch).items()
        }

    bacc.get_activation_tables = _one_set
    try:
        return _build_body(nc, reps)
    finally:
        bacc.get_activation_tables = _orig_tables


def _build_body(nc, reps: int) -> bass.Bass:
    a_d = nc.dram_tensor("alpha_logits", [B_CORE, 8], F32, kind="ExternalInput")
    b_d = nc.dram_tensor("beta_logits", [B_CORE, 8], F32, kind="ExternalInput")
    g_d = nc.dram_tensor("gamma_logits", [B_CORE, 8], F32, kind="ExternalInput")
    o_d = nc.dram_tensor("loss", [B_CORE], F32, kind="ExternalOutput")

    a_v = a_d[:].rearrange("(p n) k -> p n k", p=P)
    b_v = b_d[:].rearrange("(p n) k -> p n k", p=P)
    g_v = g_d[:].rearrange("(p n) k -> p n k", p=P)
    o_v = o_d[:].rearrange("(p n) -> p n", p=P)

    with TileContext(nc) as tc:
        import contextlib
        with tc.tile_pool(name="const", bufs=1) as constp:
            # Identity (diag 1.0) + scaled identities (diag -n) for the
            # weighted q subtraction, all fp16 stationaries.
            ident = constp.tile([P, 128], F16, tag="ident")
            masks.make_identity(nc, ident[:, :])
            nident = {}
            for n_val in (5.0, 12.0, 2.0):
                t = constp.tile([P, 128], F16, tag=f"nid{int(n_val)}")
                nc.gpsimd.memset(t, 0.0)
                nc.gpsimd.affine_select(
                    out=t, in_=t,
                    compare_op=mybir.AluOpType.not_equal,
                    fill=-n_val, base=0,
                    pattern=[[-1, 128]], channel_multiplier=1,
                )
                nident[n_val] = t

            rep_loop = tc.For_i(0, reps, 1) if reps > 1 else contextlib.nullcontext()
            with (
                rep_loop,
                tc.tile_pool(name="io", bufs=2) as io,
                tc.tile_pool(name="epool", bufs=1) as epool,
                tc.tile_pool(name="spsum", bufs=2, space="PSUM") as spsum,
                tc.tile_pool(name="lpsum", bufs=2, space="PSUM") as lpsum,
                tc.tile_pool(name="ppool", bufs=2) as ppool,
                tc.tile_pool(name="dpool", bufs=1) as dpool,
                tc.tile_pool(name="ddpool", bufs=1) as ddpool,
                tc.tile_pool(name="lnp", bufs=1) as lnp,
                tc.tile_pool(name="qpool", bufs=2) as qpool,
                tc.tile_pool(name="outp", bufs=1) as outp,
            ):
                prev = None
                for j in range(N_TILES + 1):
                    cur = j < N_TILES
                    sl = slice(j * R, (j + 1) * R)

                    last_sum_mm = None
                    if cur:
                        a_t = io.tile([P, R, 8], F32, tag="a")
                        b_t = io.tile([P, R, 8], F32, tag="b")
                        g_t = io.tile([P, R, 8], F32, tag="g")
                        # all input loads on the SP ring: one HWDGE ring
                        # saturates HBM within a DMA, and SP has nothing
                        # else to do so its SEQ absorbs the buffer waits
                        # (a load wait on the ACT ring would block exps)
                        nc.sync.dma_start(out=g_t, in_=g_v[:, sl, :])
                        nc.sync.dma_start(out=b_t, in_=b_v[:, sl, :])
                        nc.sync.dma_start(out=a_t, in_=a_v[:, sl, :])

                        # --- exps (ACT, fp16 out), in load-arrival order
                        # g, b, a. eg3 = exp(gamma[0:3]) is a SEPARATE
                        # small tile: wp reads it instead of eg, so the
                        # big eg is freed by PE's sums immediately and
                        # the next tile's exp is never blocked behind
                        # this tile's DVE chain (fp16 rounding of the
                        # same fp32 exp -> bit-identical to eg's slots).
                        ea = epool.tile([P, R, 8], F16, tag="ea")
                        eb = epool.tile([P, R, 8], F16, tag="eb")
                        eg = epool.tile([P, R, 8], F16, tag="eg")
                        eg3 = lnp.tile([P, 3, R], F16, tag="eg3")
                        nc.scalar.activation(out=eg, in_=g_t, func=AF.Exp)
                        nc.scalar.activation(
                            out=eg3, in_=_ap3(g_t, 0, [[1, 3], [8, R]]),
                            func=AF.Exp)
                        nc.scalar.activation(out=eb, in_=b_t, func=AF.Exp)
                        nc.scalar.activation(out=ea, in_=a_t, func=AF.Exp)

                        # --- softmax denominators on PE: accumulate 8
                        # identity "copy" matmuls per tensor into one
                        # [P,3,R] fp32 PSUM tile (0=sa, 1=sb, 2=sg) ---
                        s3 = spsum.tile([P, 3, R], F32, tag="s3")
                        for c, e_t in ((2, eg), (1, eb), (0, ea)):
                            for k in range(8):
                                mm = nc.tensor.matmul(
                                    s3[:, c, :], ident, e_t[:, :, k],
                                    start=(k == 0), stop=(k == 7))
                                if c == 2 and k == 7:
                                    # gamma sums end: the only part of
                                    # sums(j) the close must yield to
                                    # (it gates wp -> the whole DVE chain)
                                    last_sum_mm = mm

                    # --- PE: ALL of tile j-1's loss matmuls (30 dA/dB/dC
                    # copies + 7 weighted q + 6 dD, one PSUM accumulation).
                    # Pinned AFTER tile j's sums: they depend on DVE's
                    # maxes(j-1), and letting them precede the sums would
                    # insert that dependency into the sums->wp->S->maxes
                    # chain, turning the pipeline into a ~25us/tile cycle.
                    if prev is not None:
                        psl_p = lpsum.tile([P, R], F32, tag="loss")
                        first = True
                        for d_t in (prev["dA"], prev["dB"], prev["dC"]):
                            for t in range(10):
                                mm = nc.tensor.matmul(psl_p, ident,
                                                      d_t[:, t, :],
                                                      start=first, stop=False)
                                if first and last_sum_mm is not None:
                                    add_dep_helper(mm.ins, last_sum_mm.ins,
                                                   sync=True,
                                                   reason="pe close after sums")
                                first = False
                        for n_val, qs in _QORDER:
                            nc.tensor.matmul(psl_p, nident[n_val],
                                             prev["q16"][:, qs, :],
                                             start=False, stop=False)
                        for t in range(6):
                            nc.tensor.matmul(psl_p, ident, prev["dD"][:, t, :],
                                             start=False, stop=(t == 5))
                        prev["psl"] = psl_p

                    if not cur:
                        loss_t = outp.tile([P, R], F32, tag="loss")
                        nc.scalar.copy(out=loss_t, in_=prev["psl"])
                        nc.scalar.dma_start(out=o_v[:, prev["sl"]], in_=loss_t)
                        break

                    # Stage gamma[4:8] into the q16 pos slots NOW (DVE
                    # copy, fp32->fp16). This is the last reader of g_t:
                    # doing it early (instead of inside qpos, which sits
                    # behind ln3/wl) releases the io-g buffer immediately,
                    # so the next g load -> exp -> sums -> wp chain is not
                    # serialized behind this tile's whole DVE chain.
                    q16 = qpool.tile([P, 7, R], F16, tag="q16")
                    nc.vector.tensor_copy(
                        q16[:, 0:4, :], _ap3(g_t, 4, [[1, 4], [8, R]]))

                    # wp[g] = sg - eg3[g], g in 0..2, in place over eg3
                    # (DVE; emitted before the fused ln so ACT's wl
                    # overlaps DVE's S/q work)
                    wp = eg3
                    sg_b = _ap3(s3, 2 * R, [[0, 3], [1, R]])
                    nc.vector.tensor_sub(wp, sg_b, eg3)

                    # --- fused ln over all three sums (ACT reads PSUM) ---
                    lns = lnp.tile([P, 3, R], F16, tag="lns")
                    ln3_i = nc.scalar.activation(out=lns, in_=s3, func=AF.Ln)

                    wl = q16[:, 4:7, :]          # ln(wp) staged in q slots
                    wl_i = nc.scalar.activation(out=wl, in_=wp, func=AF.Ln)

                    # --- ACT: export tile j-1's loss (PSUM -> SBUF -> HBM
                    # on the ACT ring). Pinned AFTER ln3/wl: the copy's
                    # wait (on PE closing tile j-1) parks on the in-order
                    # ACT SEQ, and must not delay the ln chain that gates
                    # all of DVE. ---
                    if prev is not None:
                        loss_t = outp.tile([P, R], F32, tag="loss")
                        cp_i = nc.scalar.copy(out=loss_t, in_=prev["psl"])
                        add_dep_helper(cp_i.ins, ln3_i.ins, sync=True,
                                       reason="export after ln3")
                        add_dep_helper(cp_i.ins, wl_i.ins, sync=True,
                                       reason="export after wl")
                        nc.scalar.dma_start(out=o_v[:, prev["sl"]], in_=loss_t)

                    # S = lsg - lsa - lsb (DVE fp16 2x, in place over lsg)
                    s_t = lns[:, 2, :]
                    nc.vector.tensor_sub(s_t, lns[:, 2, :], lns[:, 0, :])
                    nc.vector.tensor_sub(s_t, s_t, lns[:, 1, :])

                    # q' slots, both in place at 2x:
                    #   [g4,g5,g6,g7] = staged gamma - S
                    #   [w0,w1,w2]    = wl - S
                    s_b4 = _ap3(s_t, 0, [[0, 4], [1, R]])
                    s_b3 = _ap3(s_t, 0, [[0, 3], [1, R]])
                    nc.vector.tensor_sub(q16[:, 0:4, :], q16[:, 0:4, :], s_b4)
                    nc.vector.tensor_sub(q16[:, 4:7, :], wl, s_b3)

                    # --- pair sums p16 on GpSimd (fp32 in -> fp16 out),
                    # X1 slots first so the maxes can start early ---
                    p16 = ppool.tile([P, 14, R], F16, tag="p16")
                    for a0, astr, b0, bstr, s0, n in _PAIR_BATCHES:
                        nc.gpsimd.tensor_add(
                            _ap3(p16, s0 * R, [[R, n], [1, R]]),
                            _ap3(a_t, a0, [[astr, n], [8, R]]),
                            _ap3(b_t, b0, [[bstr, n], [8, R]]))

                    # --- the 36 maxes as 9 broadcast tensor_max instrs
                    # (fp16 2x, 2 free dims max -- the 3-free-dim fused
                    # form passes CoreSim but crashes real hardware).
                    # d tiles: dA/dB/dC [P,10,R], dD [P,6,R]. ---
                    dA = dpool.tile([P, 10, R], F16, tag="dA")
                    dB = dpool.tile([P, 10, R], F16, tag="dB")
                    dC = dpool.tile([P, 10, R], F16, tag="dC")
                    dD = ddpool.tile([P, 6, R], F16, tag="dD")
                    X1 = _ap3(p16, 0, [[R, 5], [1, R]])
                    X2 = _ap3(p16, 5 * R, [[R, 5], [1, R]])

                    def qb(qs, n):
                        return _ap3(q16, qs * R, [[0, n], [1, R]])

                    # dA: [X1 vs g4 | X1 vs w1]
                    nc.vector.tensor_max(dA[:, 0:5, :], X1, qb(0, 5))
                    nc.vector.tensor_max(dA[:, 5:10, :], X1, qb(5, 5))
                    # dB: [X1 vs w2 | X2 vs g5]
                    nc.vector.tensor_max(dB[:, 0:5, :], X1, qb(6, 5))
                    nc.vector.tensor_max(dB[:, 5:10, :], X2, qb(1, 5))
                    # dC: [X2 vs w0 | X2 vs w2]
                    nc.vector.tensor_max(dC[:, 0:5, :], X2, qb(4, 5))
                    nc.vector.tensor_max(dC[:, 5:10, :], X2, qb(6, 5))
                    # dD: [p10,p11 vs g6 | p12,p13 vs g7 | p12,p13 vs w2]
                    nc.vector.tensor_max(
                        dD[:, 0:2, :], _ap3(p16, 10 * R, [[R, 2], [1, R]]),
                        qb(2, 2))
                    nc.vector.tensor_max(
                        dD[:, 2:4, :], _ap3(p16, 12 * R, [[R, 2], [1, R]]),
                        qb(3, 2))
                    nc.vector.tensor_max(
                        dD[:, 4:6, :], _ap3(p16, 12 * R, [[R, 2], [1, R]]),
                        qb(6, 2))

                    prev = {"q16": q16, "dA": dA, "dB": dB, "dC": dC,
                            "dD": dD, "sl": sl}

    nc.compile()
    return nc


_NC_CACHE = None


def _get_nc():
    global _NC_CACHE
    if _NC_CACHE is None:
        _NC_CACHE = build_kernel()
    return _NC_CACHE


def kernel(alpha_logits, beta_logits, gamma_logits, _trace=False):
    nc = _get_nc()
    in_maps = []
    for c in range(N_CORES):
        sl = slice(c * B_CORE, (c + 1) * B_CORE)
        in_maps.append({
            "alpha_logits": np.ascontiguousarray(alpha_logits[sl]),
            "beta_logits": np.ascontiguousarray(beta_logits[sl]),
            "gamma_logits": np.ascontiguousarray(gamma_logits[sl]),
        })
    res = run_bass_kernel_spmd(nc, in_maps, core_ids=list(range(N_CORES)),
                               trace=_trace)
    out = np.concatenate([r["loss"] for r in res.results])
    if _trace:
        kernel.last_result = res
    return out
